# revision 1
# baseline (speedup 1.0000x reference)
"""Trainium2 Bass kernel for a 2-layer GCN encoder (GCNConv -> ReLU -> Dropout -> GCNConv).

Strategy (8 NeuronCores, SPMD):
  - Nodes are partitioned into 8 contiguous ranges of 6250 (dst/"owned" nodes per core).
  - GCN layer is computed aggregation-first:  out = (A_hat_norm @ h) @ W + b,
    where A_hat_norm includes self-loops (folded in as extra edges).
  - Aggregation per core: edges sorted by dst block (128 dsts per block).
    For each 128-edge tile: dma_gather the 128 source rows (bf16) from the
    node table in HBM, build a selection matrix S[e, dlocal] = norm[e] via a
    fused DVE tensor_scalar (is_equal + mult against an iota matrix), and
    matmul-accumulate  S.T @ msg  into the block's PSUM tile.
  - dma_gather indices are int16, so the node table is addressed as two
    halves (rows [0, 32768) and [32768, 50000)); edges are grouped by half.
  - Layer 1 gathers directly from x (bf16); its output h2 = dropout(relu(.))
    is AllGather'd across cores to form the layer-2 gather table.
  - Dense part per block: PE-transpose of the aggregated tile, then
    matmul with the (replicated) 256x256 weight.

The dropout mask replicates jax.random.bernoulli(fold_in(key(0), 7), 0.8)
exactly (computed on host CPU with jax, as in the reference).
"""
import os
import sys
import numpy as np

_N = 50000
_E = 800000
_D = 256
_P = 128
_NC = 8
_OWN = _N // _NC          # 6250
_NB = (_OWN + _P - 1) // _P   # 49 blocks (48 full + 106)
_LO = 32768               # int16-addressable split of the node table
_GL = 8                   # max tiles (of 128 idxs) per dma_gather call
_DROP_P = 0.2

_cache = {}


def _dropout_mask():
    """Exact reproduction of the reference dropout mask, computed on CPU."""
    import jax

    cpu = jax.devices("cpu")[0]
    with jax.default_device(cpu):
        key = jax.random.fold_in(jax.random.key(0), 7)
        mask = jax.random.bernoulli(key, 1.0 - _DROP_P, (_N, _D))
        return np.asarray(mask)


def _prep_edges(edge_index: np.ndarray):
    """Host-side graph preprocessing -> per-core tile structure.

    Returns (meta, per_core) where meta has the uniform tile counts
    (n_lo[b], n_hi[b], T, NTMAX) and per_core[k] has idx/dstl/normv arrays.
    """
    src = edge_index[0].astype(np.int64)
    dst = edge_index[1].astype(np.int64)

    deg = np.bincount(dst, minlength=_N).astype(np.float32) + 1.0
    dinv = (1.0 / np.sqrt(deg.astype(np.float64))).astype(np.float32)
    enorm = dinv[src] * dinv[dst]
    snorm = dinv * dinv

    # fold self-loops in as ordinary edges
    all_src = np.concatenate([src, np.arange(_N, dtype=np.int64)])
    all_dst = np.concatenate([dst, np.arange(_N, dtype=np.int64)])
    all_w = np.concatenate([enorm, snorm]).astype(np.float32)

    core = all_dst // _OWN
    block = (all_dst % _OWN) // _P
    half = (all_src >= _LO).astype(np.int64)

    # group id per edge: (core, block, half)
    gid = (core * _NB + block) * 2 + half
    n_groups = _NC * _NB * 2
    order = np.argsort(gid, kind="stable")
    gsorted = gid[order]
    s_src = all_src[order]
    s_dst = all_dst[order]
    s_w = all_w[order]

    counts = np.bincount(gsorted, minlength=n_groups)  # [n_groups]
    cnt = counts.reshape(_NC, _NB, 2)

    # uniform tile counts across cores (program is shared by all cores)
    tiles_needed = -(-cnt // _P)  # ceil
    n_lo = tiles_needed[:, :, 0].max(axis=0)  # [NB]
    n_hi = tiles_needed[:, :, 1].max(axis=0)  # [NB]
    ntiles_bh = np.stack([n_lo, n_hi], axis=1)  # [NB, 2]
    T = int(ntiles_bh.sum())
    NTMAX = int((n_lo + n_hi).max())

    # slot offsets of each (block, half) group in the global tile stream
    tile_off = np.zeros((_NB, 2), np.int64)
    flat = ntiles_bh.reshape(-1)
    tile_off.reshape(-1)[1:] = np.cumsum(flat)[:-1]

    # per-edge destination position in the padded [T*128] stream
    grp_start = np.zeros(n_groups, np.int64)
    grp_start[1:] = np.cumsum(counts)[:-1]
    rank_in_grp = np.arange(len(gsorted)) - grp_start[gsorted]

    g_core = gsorted // (2 * _NB)
    g_rest = gsorted % (2 * _NB)
    g_block = g_rest // 2
    g_half = g_rest % 2
    pos = (tile_off[g_block, g_half] * _P) + rank_in_grp  # position in stream

    per_core = []
    for k in range(_NC):
        m = g_core == k
        idx_flat = np.zeros(T * _P, np.int16)
        dstl_flat = np.zeros(T * _P, np.float32)
        norm_flat = np.zeros(T * _P, np.float32)
        p = pos[m]
        rel = s_src[m] - np.where(g_half[m] == 1, _LO, 0)
        idx_flat[p] = rel.astype(np.int16)
        dstl_flat[p] = (s_dst[m] % _OWN % _P).astype(np.float32)
        norm_flat[p] = s_w[m]

        tiles = idx_flat.reshape(T, _P)
        # pack: position i of tile t -> [i % 16, t*8 + i//16]
        packed = tiles.reshape(T, 8, 16).transpose(2, 0, 1).reshape(16, T * 8)
        idx128 = np.tile(packed, (8, 1))
        per_core.append(
            dict(
                idx=idx128,
                dstl=dstl_flat.reshape(T, _P).T.copy(),
                normv=norm_flat.reshape(T, _P).T.copy(),
            )
        )

    meta = dict(n_lo=n_lo, n_hi=n_hi, T=T, NTMAX=NTMAX)
    return meta, per_core


def _build(meta, with_bias1, with_bias2):
    import concourse.bacc as bacc
    import concourse.mybir as mybir
    import concourse.tile as tile
    from concourse import library_config

    bf16 = mybir.dt.bfloat16
    f32 = mybir.dt.float32
    n_lo, n_hi, T, NTMAX = meta["n_lo"], meta["n_hi"], meta["T"], meta["NTMAX"]

    nc = bacc.Bacc(None, target_bir_lowering=False)
    xt_in = nc.declare_dram_parameter("xt", [_N, _D], bf16, isOutput=False)
    idx_in = nc.declare_dram_parameter("idx", [128, T * 8], mybir.dt.int16, isOutput=False)
    dstl_in = nc.declare_dram_parameter("dstl", [128, T], f32, isOutput=False)
    norm_in = nc.declare_dram_parameter("normv", [128, T], f32, isOutput=False)
    iota_in = nc.declare_dram_parameter("iota", [128, 128], bf16, isOutput=False)
    ident_in = nc.declare_dram_parameter("ident", [128, 128], bf16, isOutput=False)
    w1_in = nc.declare_dram_parameter("w1", [128, 2, _D], bf16, isOutput=False)
    w2_in = nc.declare_dram_parameter("w2", [128, 2, _D], bf16, isOutput=False)
    dmask_in = nc.declare_dram_parameter("dmask", [_OWN, _D], bf16, isOutput=False)
    if with_bias1:
        b1_in = nc.declare_dram_parameter("b1r", [128, _D], f32, isOutput=False)
    if with_bias2:
        b2_in = nc.declare_dram_parameter("b2r", [128, _D], f32, isOutput=False)
    y_out = nc.declare_dram_parameter("y", [_OWN, _D], f32, isOutput=True)

    h2_own = nc.dram_tensor("h2_own", [_OWN, _D], bf16)
    h2_full = nc.dram_tensor("h2_full", [_N, _D], bf16, addr_space="Shared")

    with tile.TileContext(nc) as tc:
        nc.gpsimd.load_library(library_config.mlp)
        with (
            tc.tile_pool(name="const", bufs=1) as cpool,
            tc.tile_pool(name="work", bufs=2) as wpool,
            tc.tile_pool(name="spool", bufs=4) as spool,
            tc.tile_pool(name="psA", bufs=2, space="PSUM") as psA,
            tc.tile_pool(name="psT", bufs=2, space="PSUM") as psT,
            tc.tile_pool(name="psO", bufs=2, space="PSUM") as psO,
        ):
            idxs = cpool.tile([128, T * 8], mybir.dt.int16)
            nc.sync.dma_start(out=idxs[:], in_=idx_in[:])
            dstl = cpool.tile([128, T], f32)
            nc.sync.dma_start(out=dstl[:], in_=dstl_in[:])
            normv = cpool.tile([128, T], f32)
            nc.sync.dma_start(out=normv[:], in_=norm_in[:])
            iota = cpool.tile([128, 128], bf16)
            nc.sync.dma_start(out=iota[:], in_=iota_in[:])
            ident = cpool.tile([128, 128], bf16)
            nc.sync.dma_start(out=ident[:], in_=ident_in[:])
            w1_sb = cpool.tile([128, 2, _D], bf16)
            nc.sync.dma_start(out=w1_sb[:], in_=w1_in[:])
            w2_sb = cpool.tile([128, 2, _D], bf16)
            nc.sync.dma_start(out=w2_sb[:], in_=w2_in[:])
            if with_bias1:
                b1_sb = cpool.tile([128, _D], f32)
                nc.sync.dma_start(out=b1_sb[:], in_=b1_in[:])
            if with_bias2:
                b2_sb = cpool.tile([128, _D], f32)
                nc.sync.dma_start(out=b2_sb[:], in_=b2_in[:])

            def layer(table, w_sb, is_first):
                g = 0
                for b in range(_NB):
                    nl, nh = int(n_lo[b]), int(n_hi[b])
                    nt = nl + nh
                    rows = min(_P, _OWN - b * _P)
                    msg = wpool.tile([128, NTMAX, _D], bf16, tag="msg")
                    for lo_t, n_t, src_ap in (
                        (0, nl, table[0:_LO, :]),
                        (nl, nh, table[_LO:_N, :]),
                    ):
                        for q0 in range(0, n_t, _GL):
                            qn = min(_GL, n_t - q0)
                            t0 = lo_t + q0
                            nc.gpsimd.dma_gather(
                                out_ap=msg[:, t0 : t0 + qn, :],
                                in_ap=src_ap,
                                idxs_ap=idxs[:, (g + t0) * 8 : (g + t0 + qn) * 8],
                                num_idxs=qn * _P,
                                num_idxs_reg=qn * _P,
                                elem_size=_D,
                            )
                    agg_ps = psA.tile([128, _D], f32, tag="agg")
                    for t in range(nt):
                        s_t = spool.tile([128, 128], bf16, tag="S")
                        nc.vector.tensor_scalar(
                            out=s_t[:],
                            in0=iota[:],
                            scalar1=dstl[:, g + t : g + t + 1],
                            scalar2=normv[:, g + t : g + t + 1],
                            op0=mybir.AluOpType.is_equal,
                            op1=mybir.AluOpType.mult,
                        )
                        nc.tensor.matmul(
                            agg_ps[:],
                            lhsT=s_t[:],
                            rhs=msg[:, t, :],
                            start=(t == 0),
                            stop=(t == nt - 1),
                        )
                    g += nt
                    agg_sb = wpool.tile([128, _D], bf16, tag="aggsb")
                    nc.vector.tensor_copy(agg_sb[:], agg_ps[:])
                    aggT_ps = psT.tile([128, _D], bf16, tag="aggT")
                    for c in range(2):
                        nc.tensor.transpose(
                            aggT_ps[:, c * 128 : (c + 1) * 128],
                            agg_sb[:, c * 128 : (c + 1) * 128],
                            ident[:],
                        )
                    aggT_sb = wpool.tile([128, _D], bf16, tag="aggTsb")
                    nc.vector.tensor_copy(aggT_sb[:], aggT_ps[:])
                    out_ps = psO.tile([128, _D], f32, tag="out")
                    for c in range(2):
                        nc.tensor.matmul(
                            out_ps[:],
                            lhsT=aggT_sb[:, c * 128 : (c + 1) * 128],
                            rhs=w_sb[:, c, :],
                            start=(c == 0),
                            stop=(c == 1),
                        )
                    r0 = b * _P
                    if is_first:
                        if with_bias1:
                            nc.vector.tensor_tensor(
                                out=out_ps[:], in0=out_ps[:], in1=b1_sb[:],
                                op=mybir.AluOpType.add,
                            )
                        dmask_t = wpool.tile([128, _D], bf16, tag="dm")
                        nc.sync.dma_start(
                            out=dmask_t[:rows], in_=dmask_in[r0 : r0 + rows, :]
                        )
                        t1 = wpool.tile([128, _D], f32, tag="t1")
                        nc.vector.tensor_tensor(
                            out=t1[:], in0=out_ps[:], in1=dmask_t[:],
                            op=mybir.AluOpType.mult,
                        )
                        h_sb = wpool.tile([128, _D], bf16, tag="h")
                        nc.vector.tensor_scalar_max(h_sb[:], t1[:], 0.0)
                        nc.sync.dma_start(
                            out=h2_own[r0 : r0 + rows, :], in_=h_sb[:rows]
                        )
                    else:
                        if with_bias2:
                            nc.vector.tensor_tensor(
                                out=out_ps[:], in0=out_ps[:], in1=b2_sb[:],
                                op=mybir.AluOpType.add,
                            )
                        y_sb = wpool.tile([128, _D], f32, tag="ysb")
                        nc.vector.tensor_copy(y_sb[:], out_ps[:])
                        nc.sync.dma_start(out=y_out[r0 : r0 + rows, :], in_=y_sb[:rows])

            layer(xt_in, w1_sb, True)
            nc.gpsimd.collective_compute(
                "AllGather",
                mybir.AluOpType.bypass,
                ins=[h2_own[:]],
                outs=[h2_full[:]],
                replica_groups=[list(range(_NC))],
            )
            layer(h2_full, w2_sb, False)
    nc.compile()
    return nc


def _get_compiled(edge_index, b1, b2):
    key = (edge_index.tobytes(), b1.tobytes(), b2.tobytes())
    import hashlib

    key = hashlib.sha1(b"".join(key)).hexdigest()
    if key in _cache:
        return _cache[key]
    meta, per_core = _prep_edges(edge_index)
    with_bias1 = bool(np.any(b1))
    with_bias2 = bool(np.any(b2))
    nc = _build(meta, with_bias1, with_bias2)
    mask = _dropout_mask()
    _cache[key] = (nc, meta, per_core, with_bias1, with_bias2, mask)
    return _cache[key]


def kernel(x, edge_index, W1, b1, W2, b2):
    import ml_dtypes
    from concourse.bass_utils import run_bass_kernel_spmd

    x = np.asarray(x)
    edge_index = np.asarray(edge_index)
    W1 = np.asarray(W1, np.float32)
    W2 = np.asarray(W2, np.float32)
    b1 = np.asarray(b1, np.float32)
    b2 = np.asarray(b2, np.float32)

    nc, meta, per_core, wb1, wb2, mask = _get_compiled(edge_index, b1, b2)

    xt = x.astype(ml_dtypes.bfloat16)
    scale = np.float32(1.0 / (1.0 - _DROP_P))
    dmask = (mask.astype(np.float32) * scale).astype(ml_dtypes.bfloat16)
    iota = np.broadcast_to(np.arange(128, dtype=np.float32), (128, 128)).astype(
        ml_dtypes.bfloat16
    )
    ident = np.eye(128, dtype=np.float32).astype(ml_dtypes.bfloat16)
    w1p = W1.reshape(2, 128, _D).transpose(1, 0, 2).astype(ml_dtypes.bfloat16)
    w2p = W2.reshape(2, 128, _D).transpose(1, 0, 2).astype(ml_dtypes.bfloat16)

    in_maps = []
    for k in range(_NC):
        m = dict(
            xt=xt,
            idx=per_core[k]["idx"],
            dstl=per_core[k]["dstl"],
            normv=per_core[k]["normv"],
            iota=iota,
            ident=ident,
            w1=w1p,
            w2=w2p,
            dmask=dmask[k * _OWN : (k + 1) * _OWN].copy(),
        )
        if wb1:
            m["b1r"] = np.broadcast_to(b1, (128, _D)).astype(np.float32).copy()
        if wb2:
            m["b2r"] = np.broadcast_to(b2, (128, _D)).astype(np.float32).copy()
        in_maps.append(m)

    res = run_bass_kernel_spmd(nc, in_maps, list(range(_NC)))
    out = np.concatenate([res.results[k]["y"] for k in range(_NC)], axis=0)
    return out.astype(np.float32)


# revision 4
# speedup vs baseline: 82.2534x; 82.2534x over previous
"""Trainium2 Bass kernel for a 2-layer GCN encoder (GCNConv -> ReLU -> Dropout -> GCNConv).

Strategy (8 NeuronCores, SPMD):
  - Nodes are partitioned into 8 contiguous ranges of 6250 (dst/"owned" nodes per core).
  - Each GCN layer is computed aggregation-first:  out = (A_hat_norm @ h) @ W + b,
    with self-loops folded in as ordinary edges.
  - Aggregation per core: edges sorted by (dst block of 128, src-half).
    For each 128-edge tile: dma_gather the 128 source rows (bf16) from the
    node table in HBM, build a selection matrix S[e, dlocal] = norm[e] via a
    fused DVE tensor_scalar (is_equal + mult against an iota matrix), and
    matmul-accumulate  S.T @ msg  into the block's PSUM tile [128 dst, 256].
  - dma_gather indices are int16, so the node table is addressed as two
    halves (rows [0, 32768) and [32768, 50000)); edges are grouped by half.
  - The x slices are AllGather'd on device into the layer-1 gather table;
    layer-1 output h2 = dropout(relu(.)) is AllGather'd for layer 2.
  - Dense part per block: PE-transpose of the aggregated tile, then matmul
    with the (replicated) 256x256 weight.

The dropout mask replicates jax.random.bernoulli(fold_in(key(0), 7), 0.8)
exactly (computed on host CPU with jax, as in the reference).
"""
import numpy as np

_N = 50000
_E = 800000
_D = 256
_P = 128
_NC = 8
_OWN = _N // _NC              # 6250
_NB = (_OWN + _P - 1) // _P   # 49 blocks (48 full + 1 of 106 rows)
_LO = 32768                   # int16-addressable split of the node table
_GL = 8                       # max tiles (of 128 idxs) per dma_gather call
_DROP_P = 0.2

_cache = {}


def _dropout_mask():
    """Exact reproduction of the reference dropout mask, computed on CPU."""
    import jax

    cpu = jax.devices("cpu")[0]
    with jax.default_device(cpu):
        key = jax.random.fold_in(jax.random.key(0), 7)
        mask = jax.random.bernoulli(key, 1.0 - _DROP_P, (_N, _D))
        return np.asarray(mask)


def _prep_edges(edge_index: np.ndarray):
    """Host-side graph preprocessing -> per-core tile structure."""
    src = edge_index[0].astype(np.int64)
    dst = edge_index[1].astype(np.int64)

    deg = np.bincount(dst, minlength=_N).astype(np.float32) + 1.0
    dinv = (1.0 / np.sqrt(deg.astype(np.float64))).astype(np.float32)
    enorm = dinv[src] * dinv[dst]
    snorm = dinv * dinv

    # fold self-loops in as ordinary edges
    all_src = np.concatenate([src, np.arange(_N, dtype=np.int64)])
    all_dst = np.concatenate([dst, np.arange(_N, dtype=np.int64)])
    all_w = np.concatenate([enorm, snorm]).astype(np.float32)

    core = all_dst // _OWN
    block = (all_dst % _OWN) // _P
    half = (all_src >= _LO).astype(np.int64)

    gid = (core * _NB + block) * 2 + half
    n_groups = _NC * _NB * 2
    order = np.argsort(gid, kind="stable")
    gsorted = gid[order]
    s_src = all_src[order]
    s_dst = all_dst[order]
    s_w = all_w[order]

    counts = np.bincount(gsorted, minlength=n_groups)
    cnt = counts.reshape(_NC, _NB, 2)

    # uniform tile counts across cores (one program for all cores)
    tiles_needed = -(-cnt // _P)
    n_lo = tiles_needed[:, :, 0].max(axis=0)
    n_hi = tiles_needed[:, :, 1].max(axis=0)
    ntiles_bh = np.stack([n_lo, n_hi], axis=1)
    T = int(ntiles_bh.sum())
    NTMAX = int((n_lo + n_hi).max())

    tile_off = np.zeros((_NB, 2), np.int64)
    flat = ntiles_bh.reshape(-1)
    tile_off.reshape(-1)[1:] = np.cumsum(flat)[:-1]

    grp_start = np.zeros(n_groups, np.int64)
    grp_start[1:] = np.cumsum(counts)[:-1]
    rank_in_grp = np.arange(len(gsorted)) - grp_start[gsorted]

    g_core = gsorted // (2 * _NB)
    g_rest = gsorted % (2 * _NB)
    g_block = g_rest // 2
    g_half = g_rest % 2
    pos = (tile_off[g_block, g_half] * _P) + rank_in_grp

    per_core = []
    for k in range(_NC):
        m = g_core == k
        idx_flat = np.zeros(T * _P, np.int16)
        dstl_flat = np.zeros(T * _P, np.float32)
        norm_flat = np.zeros(T * _P, np.float32)
        p = pos[m]
        rel = s_src[m] - np.where(g_half[m] == 1, _LO, 0)
        idx_flat[p] = rel.astype(np.int16)
        dstl_flat[p] = (s_dst[m] % _OWN % _P).astype(np.float32)
        norm_flat[p] = s_w[m]

        tiles = idx_flat.reshape(T, _P)
        # pack: position i of tile t -> [i % 16, t*8 + i//16]
        packed = tiles.reshape(T, 8, 16).transpose(2, 0, 1).reshape(16, T * 8)
        per_core.append(
            dict(
                idx=packed,
                dstl=dstl_flat.reshape(T, _P).T.copy(),
                normv=norm_flat.reshape(T, _P).T.copy(),
            )
        )

    meta = dict(n_lo=n_lo, n_hi=n_hi, T=T, NTMAX=NTMAX)
    return meta, per_core


def _build(meta, with_bias1, with_bias2):
    import concourse.bacc as bacc
    import concourse.mybir as mybir
    import concourse.tile as tile
    from concourse import library_config

    bf16 = mybir.dt.bfloat16
    f32 = mybir.dt.float32
    n_lo, n_hi, T, NTMAX = meta["n_lo"], meta["n_hi"], meta["T"], meta["NTMAX"]

    nc = bacc.Bacc(None, target_bir_lowering=False)
    xs_in = nc.declare_dram_parameter("xs", [_OWN, _D], bf16, isOutput=False)
    idx_in = nc.declare_dram_parameter("idx", [16, T * 8], mybir.dt.int16, isOutput=False)
    dstl_in = nc.declare_dram_parameter("dstl", [128, T], f32, isOutput=False)
    norm_in = nc.declare_dram_parameter("normv", [128, T], f32, isOutput=False)
    iota_in = nc.declare_dram_parameter("iota", [128, 128], bf16, isOutput=False)
    ident_in = nc.declare_dram_parameter("ident", [128, 128], bf16, isOutput=False)
    w1_in = nc.declare_dram_parameter("w1", [128, 2, _D], bf16, isOutput=False)
    w2_in = nc.declare_dram_parameter("w2", [128, 2, _D], bf16, isOutput=False)
    dmask_in = nc.declare_dram_parameter("dmask", [_OWN, _D], bf16, isOutput=False)
    if with_bias1:
        b1_in = nc.declare_dram_parameter("b1r", [128, _D], f32, isOutput=False)
    if with_bias2:
        b2_in = nc.declare_dram_parameter("b2r", [128, _D], f32, isOutput=False)
    y_out = nc.declare_dram_parameter("y", [_OWN, _D], f32, isOutput=True)

    x_own = nc.dram_tensor("x_own", [_OWN, _D], bf16)
    x_full = nc.dram_tensor("x_full", [_N, _D], bf16, addr_space="Shared")
    h2_own = nc.dram_tensor("h2_own", [_OWN, _D], bf16)
    h2_full = nc.dram_tensor("h2_full", [_N, _D], bf16, addr_space="Shared")

    with tile.TileContext(nc) as tc:
        nc.gpsimd.load_library(library_config.mlp)
        with (
            tc.tile_pool(name="const", bufs=1) as cpool,
            tc.tile_pool(name="work", bufs=2) as wpool,
            tc.tile_pool(name="spool", bufs=4) as spool,
            tc.tile_pool(name="psA", bufs=2, space="PSUM") as psA,
            tc.tile_pool(name="psT", bufs=2, space="PSUM") as psT,
            tc.tile_pool(name="psO", bufs=2, space="PSUM") as psO,
        ):
            # x slice -> internal bounce (DRAM->DRAM) -> AllGather x table
            nc.sync.dma_start(out=x_own[:], in_=xs_in[:])
            nc.gpsimd.collective_compute(
                "AllGather",
                mybir.AluOpType.bypass,
                ins=[x_own[:]],
                outs=[x_full[:]],
                replica_groups=[list(range(_NC))],
            )

            idxs = cpool.tile([128, T * 8], mybir.dt.int16)
            for c in range(8):
                nc.sync.dma_start(out=idxs[c * 16 : (c + 1) * 16, :], in_=idx_in[:])
            dstl = cpool.tile([128, T], f32)
            nc.sync.dma_start(out=dstl[:], in_=dstl_in[:])
            normv = cpool.tile([128, T], f32)
            nc.sync.dma_start(out=normv[:], in_=norm_in[:])
            iota = cpool.tile([128, 128], bf16)
            nc.sync.dma_start(out=iota[:], in_=iota_in[:])
            ident = cpool.tile([128, 128], bf16)
            nc.sync.dma_start(out=ident[:], in_=ident_in[:])
            w1_sb = cpool.tile([128, 2, _D], bf16)
            nc.sync.dma_start(out=w1_sb[:], in_=w1_in[:])
            w2_sb = cpool.tile([128, 2, _D], bf16)
            nc.sync.dma_start(out=w2_sb[:], in_=w2_in[:])
            if with_bias1:
                b1_sb = cpool.tile([128, _D], f32)
                nc.sync.dma_start(out=b1_sb[:], in_=b1_in[:])
            if with_bias2:
                b2_sb = cpool.tile([128, _D], f32)
                nc.sync.dma_start(out=b2_sb[:], in_=b2_in[:])

            def layer(table, w_sb, is_first):
                g = 0
                for b in range(_NB):
                    nl, nh = int(n_lo[b]), int(n_hi[b])
                    nt = nl + nh
                    rows = min(_P, _OWN - b * _P)
                    msg = wpool.tile([128, NTMAX, _D], bf16, tag="msg")
                    for lo_t, n_t, src_ap in (
                        (0, nl, table[0:_LO, :]),
                        (nl, nh, table[_LO:_N, :]),
                    ):
                        for q0 in range(0, n_t, _GL):
                            qn = min(_GL, n_t - q0)
                            t0 = lo_t + q0
                            nc.gpsimd.dma_gather(
                                out_ap=msg[:, t0 : t0 + qn, :],
                                in_ap=src_ap,
                                idxs_ap=idxs[:, (g + t0) * 8 : (g + t0 + qn) * 8],
                                num_idxs=qn * _P,
                                num_idxs_reg=qn * _P,
                                elem_size=_D,
                            )
                    agg_ps = psA.tile([128, _D], f32, tag="agg")
                    for t in range(nt):
                        s_t = spool.tile([128, 128], bf16, tag="S")
                        nc.vector.tensor_scalar(
                            out=s_t[:],
                            in0=iota[:],
                            scalar1=dstl[:, g + t : g + t + 1],
                            scalar2=normv[:, g + t : g + t + 1],
                            op0=mybir.AluOpType.is_equal,
                            op1=mybir.AluOpType.mult,
                        )
                        nc.tensor.matmul(
                            agg_ps[:],
                            lhsT=s_t[:],
                            rhs=msg[:, t, :],
                            start=(t == 0),
                            stop=(t == nt - 1),
                        )
                    g += nt
                    agg_sb = wpool.tile([128, _D], bf16, tag="aggsb")
                    nc.vector.tensor_copy(agg_sb[:], agg_ps[:])
                    aggT_ps = psT.tile([128, _D], bf16, tag="aggT")
                    for c in range(2):
                        nc.tensor.transpose(
                            aggT_ps[:, c * 128 : (c + 1) * 128],
                            agg_sb[:, c * 128 : (c + 1) * 128],
                            ident[:],
                        )
                    aggT_sb = wpool.tile([128, _D], bf16, tag="aggTsb")
                    nc.vector.tensor_copy(aggT_sb[:], aggT_ps[:])
                    out_ps = psO.tile([128, _D], f32, tag="out")
                    for c in range(2):
                        nc.tensor.matmul(
                            out_ps[:],
                            lhsT=aggT_sb[:, c * 128 : (c + 1) * 128],
                            rhs=w_sb[:, c, :],
                            start=(c == 0),
                            stop=(c == 1),
                        )
                    r0 = b * _P
                    if is_first:
                        if with_bias1:
                            nc.vector.tensor_tensor(
                                out=out_ps[:], in0=out_ps[:], in1=b1_sb[:],
                                op=mybir.AluOpType.add,
                            )
                        dmask_t = wpool.tile([128, _D], bf16, tag="dm")
                        nc.sync.dma_start(
                            out=dmask_t[:rows], in_=dmask_in[r0 : r0 + rows, :]
                        )
                        t1 = wpool.tile([128, _D], f32, tag="t1")
                        nc.vector.tensor_tensor(
                            out=t1[:], in0=out_ps[:], in1=dmask_t[:],
                            op=mybir.AluOpType.mult,
                        )
                        h_sb = wpool.tile([128, _D], bf16, tag="h")
                        nc.vector.tensor_scalar_max(h_sb[:], t1[:], 0.0)
                        nc.sync.dma_start(
                            out=h2_own[r0 : r0 + rows, :], in_=h_sb[:rows]
                        )
                    else:
                        if with_bias2:
                            nc.vector.tensor_tensor(
                                out=out_ps[:], in0=out_ps[:], in1=b2_sb[:],
                                op=mybir.AluOpType.add,
                            )
                        y_sb = wpool.tile([128, _D], f32, tag="ysb")
                        nc.vector.tensor_copy(y_sb[:], out_ps[:])
                        nc.sync.dma_start(out=y_out[r0 : r0 + rows, :], in_=y_sb[:rows])

            layer(x_full, w1_sb, True)
            nc.gpsimd.collective_compute(
                "AllGather",
                mybir.AluOpType.bypass,
                ins=[h2_own[:]],
                outs=[h2_full[:]],
                replica_groups=[list(range(_NC))],
            )
            layer(h2_full, w2_sb, False)
    nc.compile()
    return nc


class _Runner:
    """Cached jitted SPMD executor for a compiled Bass program.

    Mirrors concourse.bass2jax.run_bass_via_pjrt but jit-compiles once and
    keeps device-resident inputs so repeat calls measure execution only.
    """

    def __init__(self, nc):
        import jax
        import concourse.mybir as mybir
        from jax.sharding import Mesh, PartitionSpec
        from jax.experimental.shard_map import shard_map
        from concourse import bass2jax

        bass2jax.install_neuronx_cc_hook()
        self.nc = nc
        partition_name = (
            nc.partition_id_tensor.name if nc.partition_id_tensor else None
        )
        in_names, out_names, out_avals = [], [], []
        for alloc in nc.m.functions[0].allocations:
            if not isinstance(alloc, mybir.MemoryLocationSet):
                continue
            name = alloc.memorylocations[0].name
            if alloc.kind == "ExternalInput":
                if name == partition_name:
                    continue
                in_names.append(name)
            elif alloc.kind == "ExternalOutput":
                out_names.append(name)
                out_avals.append(
                    jax.core.ShapedArray(
                        tuple(alloc.tensor_shape), mybir.dt.np(alloc.dtype)
                    )
                )
        self.in_names = in_names
        self.out_names = out_names
        self.out_avals = out_avals
        n_params = len(in_names)
        n_outs = len(out_names)
        all_names = list(in_names) + list(out_names)
        if partition_name is not None:
            all_names.append(partition_name)

        def _body(*args):
            operands = list(args)
            if partition_name is not None:
                operands.append(bass2jax.partition_id_tensor())
            outs = bass2jax._bass_exec_p.bind(
                *operands,
                out_avals=tuple(out_avals),
                in_names=tuple(all_names),
                out_names=tuple(out_names),
                lowering_input_output_aliases=(),
                sim_require_finite=True,
                sim_require_nnan=True,
                nc=nc,
            )
            return tuple(outs)

        devices = jax.devices()[:_NC]
        self.mesh = Mesh(np.asarray(devices), ("core",))
        in_specs = (PartitionSpec("core"),) * (n_params + n_outs)
        out_specs = (PartitionSpec("core"),) * n_outs
        self.jitted = jax.jit(
            shard_map(
                _body,
                mesh=self.mesh,
                in_specs=in_specs,
                out_specs=out_specs,
                check_rep=False,
            ),
            donate_argnums=tuple(range(n_params, n_params + n_outs)),
            keep_unused=True,
        )
        self._dev_inputs = None
        self._dev_key = None

    def put_inputs(self, global_arrays: dict):
        """global_arrays[name] has shape (NC*per_core0, ...)."""
        import jax
        from jax.sharding import NamedSharding, PartitionSpec

        sh = NamedSharding(self.mesh, PartitionSpec("core"))
        self._dev_inputs = [
            jax.device_put(global_arrays[n], sh) for n in self.in_names
        ]

    def zeros_outs(self):
        import jax.numpy as jnp
        from jax.sharding import NamedSharding, PartitionSpec

        sh = NamedSharding(self.mesh, PartitionSpec("core"))
        import jax

        return [
            jax.device_put(
                jnp.zeros((_NC * a.shape[0], *a.shape[1:]), a.dtype), sh
            )
            for a in self.out_avals
        ]

    def run(self):
        outs = self.jitted(*self._dev_inputs, *self.zeros_outs())
        return dict(zip(self.out_names, outs))


def _get_compiled(edge_index, b1, b2):
    import hashlib

    key = hashlib.sha1(
        edge_index.tobytes() + b1.tobytes() + b2.tobytes()
    ).hexdigest()
    if key in _cache:
        return _cache[key]
    meta, per_core = _prep_edges(edge_index)
    with_bias1 = bool(np.any(b1))
    with_bias2 = bool(np.any(b2))
    nc = _build(meta, with_bias1, with_bias2)
    mask = _dropout_mask()
    runner = _Runner(nc)
    _cache[key] = (runner, meta, per_core, with_bias1, with_bias2, mask)
    return _cache[key]


def _global_inputs(x, W1, b1, W2, b2, per_core, wb1, wb2, mask):
    import ml_dtypes

    xt = x.astype(ml_dtypes.bfloat16)
    scale = np.float32(1.0 / (1.0 - _DROP_P))
    dmask = (mask.astype(np.float32) * scale).astype(ml_dtypes.bfloat16)
    iota = np.broadcast_to(np.arange(128, dtype=np.float32), (128, 128)).astype(
        ml_dtypes.bfloat16
    )
    ident = np.eye(128, dtype=np.float32).astype(ml_dtypes.bfloat16)
    w1p = W1.reshape(2, 128, _D).transpose(1, 0, 2).astype(ml_dtypes.bfloat16)
    w2p = W2.reshape(2, 128, _D).transpose(1, 0, 2).astype(ml_dtypes.bfloat16)

    g = {
        "xs": xt,  # [N, D] -> row-sharded slices
        "dmask": dmask,  # [N, D]
        "idx": np.concatenate([pc["idx"] for pc in per_core], axis=0),
        "dstl": np.concatenate([pc["dstl"] for pc in per_core], axis=0),
        "normv": np.concatenate([pc["normv"] for pc in per_core], axis=0),
        "iota": np.tile(iota, (_NC, 1)),
        "ident": np.tile(ident, (_NC, 1)),
        "w1": np.tile(w1p, (_NC, 1, 1)),
        "w2": np.tile(w2p, (_NC, 1, 1)),
    }
    if wb1:
        g["b1r"] = np.tile(
            np.broadcast_to(b1, (128, _D)).astype(np.float32), (_NC, 1)
        )
    if wb2:
        g["b2r"] = np.tile(
            np.broadcast_to(b2, (128, _D)).astype(np.float32), (_NC, 1)
        )
    return g


def kernel(x, edge_index, W1, b1, W2, b2):
    x = np.asarray(x, np.float32)
    edge_index = np.asarray(edge_index)
    W1 = np.asarray(W1, np.float32)
    W2 = np.asarray(W2, np.float32)
    b1 = np.asarray(b1, np.float32)
    b2 = np.asarray(b2, np.float32)

    runner, meta, per_core, wb1, wb2, mask = _get_compiled(edge_index, b1, b2)
    g = _global_inputs(x, W1, b1, W2, b2, per_core, wb1, wb2, mask)
    runner.put_inputs(g)
    outs = runner.run()
    y = np.asarray(outs["y"])  # [N, D] already in node order
    return y.astype(np.float32)
